# revision 4
# baseline (speedup 1.0000x reference)
"""Multi-head self-attention (pre-LN, residual) Trainium2 Bass kernel.

Problem: B=4, S=2048, D=128, H=4, Dh=32, fp32 -> rel err ~1.2e-3.
Sharding: 8 cores = 4 batches x 2 query-halves (1024 queries/core).
Each core receives its batch's full x, row-shuffled by the host so that
(a) the core's query half occupies device positions 0..1023 (attention is
permutation-invariant over keys) and (b) each SBUF partition loads
consecutive DRAM rows (8KB-contiguous DMA chunks at full bandwidth).

Fully transposed dataflow ([feature, seq] layouts) so the softmax
reduction rides the PE and no giant P-matrix transpose is needed:
  xn0^T --W--> Q^T,K^T [hd, s] bf16;  V [s, hd] bf16
  S^T[k,q] = K^T.T @ Q^T     2+2 heads packed via PE row-tiling (K=32)
  P = exp(S^T - 8)           split across ACT (table exp) and DVE
                             (Schraudolph int16 tensor_scalar whose bits
                             are bf16 exp) by COLUMN so both engines run
                             just under the pipeline rate
  ctx^T[hd,q] = V.T @ P      4 heads packed via PE col-tiling (M=32)
  den[hd,q]   = 1.T @ P      col-tiled ones-matmul (per-head row blocks)
  out^T = Wo.T @ (ctx^T * recip_approx(den)) + (x^T + bias)

The steady-state pipeline limiter is the exp work (ACT+DVE), so this
version keeps both ALU engines loaded evenly and keeps ACT pinned to the
exp_and_others table set for the whole kernel (no Sqrt -> no ~2.7us
ACT_TABLE_LOAD thrash): LN rsqrt is a Quake-style bit trick + 1 Newton
step on DVE, K/Q bias adds ride ACT Identity (bias AP), LN stats use one
grouped bn_stats per 4 tiles + a manual Welford merge, xn0 runs on the
otherwise-idle GPSIMD, PSUM->SBUF copies are batched 4 tiles per op, and
DMA descriptors issue from the GPSIMD queue instead of ACT. The exp
table is preloaded via a dummy exp during the input DMA. Dummy
full-array matmul bursts warm the PE HAM clock-gate; prep blocks and the
chunk-0 tail are interleaved into the attention k-loops so the PE never
idles long enough for the HAM MID window to re-throttle it.
"""

import sys

if "/opt/trn_rl_repo" not in sys.path:
    sys.path.insert(0, "/opt/trn_rl_repo")

import numpy as np

import concourse.bacc as bacc
import concourse.tile as tile
import concourse.mybir as mybir
from concourse.bass_utils import run_bass_kernel_spmd
from concourse.masks import make_identity

F32 = mybir.dt.float32
F32R = mybir.dt.float32r
BF16 = mybir.dt.bfloat16
I16 = mybir.dt.int16
I32 = mybir.dt.int32
AF = mybir.ActivationFunctionType
OP = mybir.AluOpType

B, S, D = 4, 2048, 128
H, DH = 4, 32
N_CORES = 8
QH = S // 2  # queries per core
NT = S // 16 // 8  # 16 s-tiles
NT = S // 128
NQT = QH // 128
CHUNK = 512
NCH = QH // CHUNK
NKT = S // 128
EPS = 1e-6
ISQ = 1.0 / np.sqrt(np.float32(DH))
SHIFT = 8.0
# Schraudolph bf16 exp: int16(x*SA + SB).bits == bf16(exp(x - SHIFT))
SA = float(128.0 / np.log(2.0))
SB = float(127.0 * 128.0 - 0.0579 * 128.0 - SHIFT * 128.0 / np.log(2.0))
# Quake rsqrt magic as float (int bits 0x5f3759df)
MAGICF = float(0x5F3759DF)

GROUPS = ((0, 2), (1, 3))  # (A on ACT, B on DVE); same-parity heads share
# a ctxden bank so Wo row masks stay partition-aligned.

# columns of group B's exp handled by ACT table-exp (rest on DVE
# Schraudolph); tunes the ACT/DVE balance.
EB = 224

_compiled = None


def _build():
    nc = bacc.Bacc(
        "TRN2",
        target_bir_lowering=False,
        debug=False,
        enable_asserts=False,
        num_devices=N_CORES,
    )

    xkv_d = nc.dram_tensor("xkv", [S, D], F32, kind="ExternalInput").ap()
    wq_d = nc.dram_tensor("wq", [D, D], F32, kind="ExternalInput").ap()
    wk_d = nc.dram_tensor("wk", [D, D], F32, kind="ExternalInput").ap()
    wv_d = nc.dram_tensor("wv", [D, D], F32, kind="ExternalInput").ap()
    wo_d = nc.dram_tensor("wo", [D, D], F32, kind="ExternalInput").ap()
    # rows: gamma, beta, bq, bk, bv, bo
    vecs_d = nc.dram_tensor("vecs", [6, D], F32, kind="ExternalInput").ap()
    outT_d = nc.dram_tensor("outT", [D, QH], F32, kind="ExternalOutput").ap()

    with tile.TileContext(nc) as tc:
        consts = tc.alloc_tile_pool(name="consts", bufs=1)
        sbW = tc.alloc_tile_pool(name="sbW", bufs=1)
        sbBig = tc.alloc_tile_pool(name="sbBig", bufs=1)
        sbTmp = tc.alloc_tile_pool(name="sbTmp", bufs=3)

        # warmup sources first so the PE warmup chain starts immediately
        wsrc = consts.tile([128, 512], BF16)
        nc.vector.memset(wsrc, 0.5)
        wones = consts.tile([128, DH], BF16)
        nc.vector.memset(wones, 1.0)
        nshift = consts.tile([128, 1], F32)
        nc.vector.memset(nshift, -SHIFT)
        # preload the exp table on ACT while input DMAs are in flight
        tabscratch = consts.tile([128, 1], F32)
        nc.scalar.activation(tabscratch, nshift, AF.Exp, bias=nshift, scale=1.0)
        ident = consts.tile([128, 128], F32)
        make_identity(nc, ident)

        # ---- input DMAs (issued from the idle GPSIMD queue, not ACT) ----
        wq_raw = sbW.tile([D, D], F32)
        wk_raw = sbW.tile([D, D], F32)
        wv_raw = sbW.tile([D, D], F32)
        wo_raw = sbW.tile([D, D], F32)
        nc.gpsimd.dma_start(out=wq_raw, in_=wq_d)
        nc.gpsimd.dma_start(out=wk_raw, in_=wk_d)
        nc.gpsimd.dma_start(out=wv_raw, in_=wv_d)
        nc.gpsimd.dma_start(out=wo_raw, in_=wo_d)
        smallT = sbW.tile([D, 6], F32)  # cols: gamma,beta,bq,bk,bv,bo
        nc.gpsimd.dma_start(out=smallT, in_=vecs_d.rearrange("v d -> d v"))

        xkv_sb = sbBig.tile([128, NT, 128], F32)
        xkv_r = xkv_d.rearrange("(p t) d -> p t d", t=NT)
        for c4 in range(4):
            nc.sync.dma_start(
                out=xkv_sb[:, c4 * 4 : (c4 + 1) * 4, :],
                in_=xkv_r[:, c4 * 4 : (c4 + 1) * 4, :],
            )

        ps_a = tc.alloc_tile_pool(name="ps_a", bufs=2, space="PSUM")

        # HAM warm-up chain (independent; fills PE during DVE/DMA setup)
        for _ in range(12):
            wps = ps_a.tile([128, 512], F32, name="wps", tag="a")
            nc.tensor.matmul(wps[0:DH, :], wones, wsrc, start=True, stop=True)

        def warm(n):
            for _ in range(n):
                wp = ps_a.tile([128, 512], F32, name="warmx", tag="a")
                nc.tensor.matmul(wp[0:DH, :], wones, wsrc, start=True, stop=True)

        # ---- fold gamma/biases into projection weights ----
        gam = smallT[:, 0:1]
        bet = smallT[:, 1:2]
        gq = sbW.tile([128, 1], F32)
        nc.vector.tensor_scalar_mul(gq, gam, float(ISQ))
        wq_f = sbW.tile([D, D], F32R)
        wk_f = sbW.tile([D, D], F32R)
        wv_f = sbW.tile([D, D], F32R)
        nc.vector.tensor_scalar_mul(wq_f, wq_raw, gq)
        nc.vector.tensor_scalar_mul(wk_f, wk_raw, gam)
        nc.vector.tensor_scalar_mul(wv_f, wv_raw, gam)

        wo_r = sbW.tile([D, D], F32R)
        nc.vector.tensor_copy(wo_r, wo_raw)
        bqe = sbW.tile([128, 1], F32)
        bke = sbW.tile([128, 1], F32)
        bve = sbW.tile([128, 1], F32)
        rbias = sbW.tile([128, 1], F32)
        t_ps = ps_a.tile([128, 1], F32, tag="a")
        nc.tensor.matmul(t_ps, wq_raw, bet, start=True, stop=True)
        nc.vector.tensor_scalar(
            bqe, t_ps, smallT[:, 2:3], float(ISQ), op0=OP.add, op1=OP.mult
        )
        t_ps = ps_a.tile([128, 1], F32, tag="a")
        nc.tensor.matmul(t_ps, wk_raw, bet, start=True, stop=True)
        nc.vector.tensor_scalar_add(bke, t_ps, smallT[:, 3:4])
        t_ps = ps_a.tile([128, 1], F32, tag="a")
        nc.tensor.matmul(t_ps, wv_raw, bet, start=True, stop=True)
        nc.vector.tensor_scalar_add(bve, t_ps, smallT[:, 4:5])
        t_ps = ps_a.tile([128, 1], F32, tag="a")
        nc.tensor.matmul(t_ps, wo_raw, bve, start=True, stop=True)
        nc.vector.tensor_scalar_add(rbias, t_ps, smallT[:, 5:6])

        # ---- LayerNorm + transposes + projections, pipelined with the
        # attention loop. Block b covers s-tiles 4b..4b+3: grouped bn_stats,
        # Welford merge + bit-trick rsqrt on DVE, xn0 on GPSIMD, transposes
        # on PE (4 into one PSUM tile, one batched ACT copy out), K/Q
        # projection bias-adds on ACT Identity.
        xn0_sb = sbBig.tile([128, NT, 128], F32)
        xkvT = sbBig.tile([128, S], F32R)  # xn0^T [d, s]
        kT = sbBig.tile([128, S], BF16)
        qT = sbBig.tile([128, QH], BF16)
        v_sb = sbBig.tile([128, NT, 128], BF16)
        residT = sbBig.tile([128, QH], F32)  # x^T + resid_bias (query half)

        def prep_block(b4):
            sl4 = slice(b4 * 4, b4 * 4 + 4)
            stats = sbTmp.tile([128, 4, 6], F32, tag="st")
            for j, t in enumerate(range(b4 * 4, b4 * 4 + 4)):
                nc.vector.bn_stats(stats[:, j, :], xkv_sb[:, t, :])
            # Welford merge of bn_stats' even/odd element halves:
            # mean = m_e - 0.5*(m_e - m_o); M2 = M2e + M2o + 32*(m_e-m_o)^2
            delta = sbTmp.tile([128, 4], F32, tag="d0")
            mean4 = sbTmp.tile([128, 4], F32, tag="d1")
            m2s = sbTmp.tile([128, 4], F32, tag="d2")
            lnv = sbTmp.tile([128, 4], F32, tag="d3")
            rs4 = sbTmp.tile([128, 4], F32, tag="d4")
            nc.vector.tensor_tensor(
                delta, stats[:, :, 1], stats[:, :, 4], op=OP.subtract
            )
            nc.vector.scalar_tensor_tensor(
                mean4, delta, -0.5, stats[:, :, 1], op0=OP.mult, op1=OP.add
            )
            nc.vector.tensor_tensor(m2s, stats[:, :, 2], stats[:, :, 5], op=OP.add)
            # lnv = (M2e+M2o + 32*delta^2)/128 + eps  (two fused steps)
            nc.vector.tensor_tensor(delta, delta, delta, op=OP.mult)
            nc.vector.scalar_tensor_tensor(
                m2s, delta, 32.0, m2s, op0=OP.mult, op1=OP.add
            )
            nc.vector.tensor_scalar(
                lnv, m2s, float(1.0 / 128.0), EPS, op0=OP.mult, op1=OP.add
            )
            # rs = rsqrt(lnv): bit-trick seed + 1 Newton step
            nc.vector.tensor_scalar(
                rs4.bitcast(I32), lnv.bitcast(I32), -0.5, MAGICF,
                op0=OP.mult, op1=OP.add,
            )
            y2 = sbTmp.tile([128, 4], F32, tag="d5")
            nc.vector.tensor_tensor(y2, rs4, rs4, op=OP.mult)
            nc.vector.tensor_tensor(y2, lnv, y2, op=OP.mult)
            nc.vector.tensor_scalar(y2, y2, -0.5, 1.5, op0=OP.mult, op1=OP.add)
            nc.vector.tensor_tensor(rs4, rs4, y2, op=OP.mult)
            for j, t in enumerate(range(b4 * 4, b4 * 4 + 4)):
                nc.vector.tensor_scalar(
                    xn0_sb[:, t, :],
                    xkv_sb[:, t, :],
                    mean4[:, j : j + 1],
                    rs4[:, j : j + 1],
                    op0=OP.subtract,
                    op1=OP.mult,
                )
            ps_x = ps_a.tile([128, 512], F32, name="ps_x", tag="a")
            for j, t in enumerate(range(b4 * 4, b4 * 4 + 4)):
                nc.tensor.transpose(
                    ps_x[:, j * 128 : (j + 1) * 128], xn0_sb[:, t, :], ident
                )
            c = b4
            nc.scalar.copy(xkvT[:, c * CHUNK : (c + 1) * CHUNK], ps_x)
            pp = ps_a.tile([128, CHUNK], F32, name="ps_k", tag="a")
            nc.tensor.matmul(
                pp, wk_f, xkvT[:, c * CHUNK : (c + 1) * CHUNK], start=True, stop=True
            )
            nc.scalar.activation(
                kT[:, c * CHUNK : (c + 1) * CHUNK], pp, AF.Identity,
                bias=bke, scale=1.0,
            )
            if c < NCH:
                pp = ps_a.tile([128, CHUNK], F32, name="ps_q", tag="a")
                nc.tensor.matmul(
                    pp, wq_f, xkvT[:, c * CHUNK : (c + 1) * CHUNK],
                    start=True, stop=True,
                )
                nc.scalar.activation(
                    qT[:, c * CHUNK : (c + 1) * CHUNK], pp, AF.Identity,
                    bias=bqe, scale=1.0,
                )
            ps_v = ps_a.tile([128, 512], F32, name="ps_v", tag="a")
            for j, t in enumerate(range(b4 * 4, b4 * 4 + 4)):
                nc.tensor.matmul(
                    ps_v[:, j * 128 : (j + 1) * 128],
                    xkvT[:, t * 128 : (t + 1) * 128],
                    wv_f,
                    start=True,
                    stop=True,
                )
            nc.scalar.copy(v_sb[:, sl4, :], ps_v)

        def resid_block(b8):
            # 4 raw-x transposes into one PSUM tile, one biased ACT copy out
            ps_r = ps_a.tile([128, 512], F32, name="ps_r", tag="a")
            for j, t in enumerate(range(b8 * 4, b8 * 4 + 4)):
                nc.tensor.transpose(
                    ps_r[:, j * 128 : (j + 1) * 128], xkv_sb[:, t, :], ident
                )
            nc.scalar.activation(
                residT[:, b8 * 512 : (b8 + 1) * 512], ps_r, AF.Identity,
                bias=rbias, scale=1.0,
            )

        # ---- attention (interleaved with prep blocks) ----
        ps_e = tc.alloc_tile_pool(name="ps_e", bufs=1, space="PSUM")
        pPool = tc.alloc_tile_pool(name="pPool", bufs=6)

        ctx_sb = sbBig.tile([128, CHUNK], F32)
        den_all = sbBig.tile([128, CHUNK], F32)
        ctx_ps = None
        den_ps = None

        def attn_scores(qc, kt):
            q0 = qc * CHUNK
            k0 = kt * 128
            p_sb = [None, None]
            sps = []
            for g, heads in enumerate(GROUPS):
                sp = ps_e.tile([128, 2 * CHUNK], F32, name=f"s{g}", tag="s", bufs=2)
                for i, h in enumerate(heads):
                    nc.tensor.matmul(
                        sp[:, i * CHUNK : (i + 1) * CHUNK],
                        kT[h * DH : (h + 1) * DH, k0 : k0 + 128],
                        qT[h * DH : (h + 1) * DH, q0 : q0 + CHUNK],
                        start=True,
                        stop=True,
                        tile_position=(h * DH, 0),
                    )
                sps.append(sp)
            # group A: ACT table exp, bf16 out
            pA = pPool.tile([128, 2 * CHUNK], BF16, tag="p")
            nc.scalar.activation(pA, sps[0], AF.Exp, bias=nshift, scale=1.0)
            p_sb[0] = pA
            # group B: first EB columns on ACT, rest on DVE Schraudolph
            pB = pPool.tile([128, 2 * CHUNK], I16, tag="p")
            if EB > 0:
                nc.scalar.activation(
                    pB.bitcast(BF16)[:, 0:EB], sps[1][:, 0:EB], AF.Exp,
                    bias=nshift, scale=1.0,
                )
            nc.vector.tensor_scalar(
                pB[:, EB:], sps[1][:, EB:], SA, SB, op0=OP.mult, op1=OP.add
            )
            p_sb[1] = pB.bitcast(BF16)
            return p_sb

        def attn_ctxden(qc, kt, p_sb):
            for g, heads in enumerate(GROUPS):
                for i, h in enumerate(heads):
                    nc.tensor.matmul(
                        ctx_ps[h * DH : (h + 1) * DH, :],
                        v_sb[:, kt, h * DH : (h + 1) * DH],
                        p_sb[g][:, i * CHUNK : (i + 1) * CHUNK],
                        start=(kt == 0),
                        stop=(kt == NKT - 1),
                        tile_position=(0, h * DH),
                    )
            for g, heads in enumerate(GROUPS):
                for i, h in enumerate(heads):
                    nc.tensor.matmul(
                        den_ps[h * DH : (h + 1) * DH, :],
                        wones,
                        p_sb[g][:, i * CHUNK : (i + 1) * CHUNK],
                        start=(kt == 0),
                        stop=(kt == NKT - 1),
                        tile_position=(0, h * DH),
                    )

        def chunk_tail(qc, ctx_src, den_recip):
            q0 = qc * CHUNK
            ctxn = sbTmp.tile([128, CHUNK], F32R, tag="cn")
            nc.vector.tensor_tensor(ctxn, ctx_src, den_recip, op=OP.mult)
            out_ps = ps_a.tile([128, CHUNK], F32, name="out_ps", tag="a")
            nc.tensor.matmul(out_ps, wo_r, ctxn, start=True, stop=True)
            fin = sbTmp.tile([128, CHUNK], F32, tag="fin")
            nc.vector.tensor_add(fin, out_ps, residT[:, q0 : q0 + CHUNK])
            nc.sync.dma_start(out=outT_d[:, q0 : q0 + CHUNK], in_=fin)

        prep_block(0)

        # chunk 0: prep blocks 1-3 and HAM keep-alives injected into the
        # k-loop; scores+exp emitted one ktile ahead of ctx/den so the
        # in-order PE stream always has runnable work while exps are in
        # flight.
        ctx_ps = ps_e.tile([128, CHUNK], F32, name="ctx0", tag="ctx")
        den_ps = ps_e.tile([128, CHUNK], F32, name="den0", tag="den")
        pending = attn_scores(0, 0)
        for kt in range(NKT):
            if kt == 1:
                prep_block(1)
            elif kt == 4:
                prep_block(2)
            elif kt == 8:
                prep_block(3)
            elif kt in (2, 6, 12):
                warm(1)
            nxt = attn_scores(0, kt + 1) if kt + 1 < NKT else None
            attn_ctxden(0, kt, pending)
            pending = nxt
        nc.vector.tensor_copy(ctx_sb, ctx_ps)
        nc.vector.reciprocal_approx_fast(den_all, den_ps)

        # chunk 1 (resid transposes + chunk-0 tail injected into the loop)
        ctx_ps = ps_e.tile([128, CHUNK], F32, name="ctx1", tag="ctx")
        den_ps = ps_e.tile([128, CHUNK], F32, name="den1", tag="den")
        warm(2)
        pending = attn_scores(1, 0)
        for kt in range(NKT):
            if kt == 2:
                resid_block(0)
            elif kt == 6:
                resid_block(1)
            elif kt == 10:
                chunk_tail(0, ctx_sb, den_all)
            elif kt in (4, 13):
                warm(1)
            nxt = attn_scores(1, kt + 1) if kt + 1 < NKT else None
            attn_ctxden(1, kt, pending)
            pending = nxt
        den1r = sbTmp.tile([128, CHUNK], F32, tag="dr")
        nc.vector.reciprocal_approx_fast(den1r, den_ps)
        chunk_tail(1, ctx_ps, den1r)

        pPool.release()
        ps_e.release()
        ps_a.release()
        sbTmp.release()
        sbBig.release()
        sbW.release()
        consts.release()

    nc.compile()
    return nc


def _get_compiled():
    global _compiled
    if _compiled is None:
        _compiled = _build()
    return _compiled


# device position j <- host row (j%128)*16 + j//128
_DEV2HOST = (np.arange(S) % 128) * NT + np.arange(S) // 128
_HOSTPERM = np.empty(S, dtype=np.int64)
_HOSTPERM[_DEV2HOST] = np.arange(S)


def kernel(x, Wq, bq, Wk, bk, Wv, bv, gamma, beta, Wo, bo):
    x = np.asarray(x, dtype=np.float32)
    vecs = np.stack(
        [np.asarray(a, dtype=np.float32) for a in (gamma, beta, bq, bk, bv, bo)]
    )
    wq = np.ascontiguousarray(np.asarray(Wq, dtype=np.float32))
    wk = np.ascontiguousarray(np.asarray(Wk, dtype=np.float32))
    wv = np.ascontiguousarray(np.asarray(Wv, dtype=np.float32))
    wo = np.ascontiguousarray(np.asarray(Wo, dtype=np.float32))

    nc = _get_compiled()

    in_maps = []
    for c in range(N_CORES):
        b, half = c // 2, c % 2
        off = half * QH
        xroll = np.roll(x[b], -off, axis=0)
        xin = np.ascontiguousarray(xroll[_HOSTPERM])
        in_maps.append(
            {"xkv": xin, "wq": wq, "wk": wk, "wv": wv, "wo": wo, "vecs": vecs}
        )

    res = run_bass_kernel_spmd(nc, in_maps, core_ids=list(range(N_CORES)), trace=False)

    out = np.empty((B, S, D), dtype=np.float32)
    for c in range(N_CORES):
        b, half = c // 2, c % 2
        off = half * QH
        out[b, off : off + QH, :] = res.results[c]["outT"].T
    return out


# revision 18
# speedup vs baseline: 1.6802x; 1.6802x over previous
"""Multi-head self-attention (pre-LN, residual) Trainium2 Bass kernel.

Problem: B=4, S=2048, D=128, H=4, Dh=32, fp32 -> rel err ~1.5e-3.
Sharding: 8 cores = 4 batches x 2 query-halves (1024 queries/core).
Each core receives its batch's full x, row-shuffled by the host so that
(a) the core's query half occupies device positions 0..1023 (attention is
permutation-invariant over keys) and (b) each SBUF partition loads
consecutive DRAM rows (8KB-contiguous DMA chunks at full bandwidth).

Fully transposed dataflow ([feature, seq] layouts) so the softmax
reduction rides the PE and no giant P-matrix transpose is needed:
  xn0^T --W--> Q^T,K^T [hd, s] bf16;  V [s, hd] bf16
  S^T[k,q] = K^T.T @ Q^T     2+2 heads packed via PE row-tiling (K=32)
  P_A = exp(S^T - 8)         heads {0,2} on ACT (table exp, bf16 out)
  P_B = schraudolph(S^T - 8) heads {1,3} on DVE: ONE tensor_scalar
                             (x*SA+SB) with int16 convert-on-write whose
                             bits are bf16 exp (min-RMS corrected, ~2%)
  ctx^T[hd,q] = V.T @ P      4 heads packed via PE col-tiling (M=32)
  den[hd,q]   = 1.T @ P      col-tiled ones-matmul (per-head row blocks)
  out^T = Wo.T @ (ctx^T * recip_approx(den)) + (x^T + bias)

The per-ktile critical cycle is scores[PE] -> exp[ACT/DVE] -> frees the
scores PSUM buffer -> next scores[PE]; with only 2 score buffers the exp
latency serializes into that cycle, the PE goes gappy, and the HAM
clock-gate re-throttles it to 1.2 GHz. So ALL rotating PSUM (scores,
prep, warmups, tails) shares one 3-buffer pool (6 banks) + dedicated
ctx/den accumulator banks: scores run 1.5 ktiles ahead and the exp
latency is fully hidden. ACT stays pinned to the exp_and_others table
set the whole kernel (exp preloaded during the input DMA; K/Q bias adds
and half of xn0 ride ACT Identity with bias/scale APs; LN rsqrt is a
Quake bit-trick + 1 Newton step on DVE; no Sqrt -> no ~2.7us table
thrash). LN stats finalize via a manual Welford merge of bn_stats'
even/odd halves. PSUM->SBUF copies are batched 4 s-tiles per op, DMA
descriptors issue from the idle GPSIMD queue, and dummy full-array
matmul bursts warm the HAM clock-gate through the LN ramp.
"""

import sys

if "/opt/trn_rl_repo" not in sys.path:
    sys.path.insert(0, "/opt/trn_rl_repo")

import numpy as np

import concourse.bacc as bacc
import concourse.tile as tile
import concourse.mybir as mybir
from concourse.bass_utils import run_bass_kernel_spmd
from concourse.masks import make_identity

F32 = mybir.dt.float32
F32R = mybir.dt.float32r
BF16 = mybir.dt.bfloat16
I16 = mybir.dt.int16
I32 = mybir.dt.int32
AF = mybir.ActivationFunctionType
OP = mybir.AluOpType

B, S, D = 4, 2048, 128
H, DH = 4, 32
N_CORES = 8
QH = S // 2  # queries per core
NT = S // 128  # 16 s-tiles
NQT = QH // 128
CHUNK = 512
NCH = QH // CHUNK
NKT = S // 128
EPS = 1e-6
ISQ = 1.0 / np.sqrt(np.float32(DH))
SHIFT = 8.0
# Schraudolph bf16 exp: int16(x*SA + SB).bits == bf16(exp(x - SHIFT))
SA = float(128.0 / np.log(2.0))
SB = float(127.0 * 128.0 - 0.0579 * 128.0 - SHIFT * 128.0 / np.log(2.0))
# Quake rsqrt magic as float (int bits 0x5f3759df)
MAGICF = float(0x5F3759DF)

GROUPS = ((0, 2), (1, 3))  # (A on ACT, B on DVE); same-parity heads share
# a ctxden bank so Wo row masks stay partition-aligned.

_compiled = None


def _build():
    nc = bacc.Bacc(
        "TRN2",
        target_bir_lowering=False,
        debug=False,
        enable_asserts=False,
        num_devices=N_CORES,
    )

    xkv_d = nc.dram_tensor("xkv", [S, D], F32, kind="ExternalInput").ap()
    wq_d = nc.dram_tensor("wq", [D, D], F32, kind="ExternalInput").ap()
    wk_d = nc.dram_tensor("wk", [D, D], F32, kind="ExternalInput").ap()
    wv_d = nc.dram_tensor("wv", [D, D], F32, kind="ExternalInput").ap()
    wo_d = nc.dram_tensor("wo", [D, D], F32, kind="ExternalInput").ap()
    # rows: gamma, beta, bq, bk, bv, bo
    vecs_d = nc.dram_tensor("vecs", [D, 6], F32, kind="ExternalInput").ap()
    outT_d = nc.dram_tensor("outT", [D, QH], F32, kind="ExternalOutput").ap()

    with tile.TileContext(nc) as tc:
        consts = tc.alloc_tile_pool(name="consts", bufs=1)
        sbW = tc.alloc_tile_pool(name="sbW", bufs=1)
        sbBig = tc.alloc_tile_pool(name="sbBig", bufs=1)
        sbTmp = tc.alloc_tile_pool(name="sbTmp", bufs=3)

        # warmup sources first so the PE warmup chain starts immediately
        wsrc = consts.tile([128, 512], BF16)
        nc.vector.memset(wsrc, 0.5)
        wones = consts.tile([128, DH], BF16)
        nc.vector.memset(wones, 1.0)
        nshift = consts.tile([128, 1], F32)
        nc.vector.memset(nshift, -SHIFT)
        # preload the exp table on ACT while input DMAs are in flight
        tabscratch = consts.tile([128, 1], F32)
        nc.scalar.activation(tabscratch, nshift, AF.Exp, bias=nshift, scale=1.0)
        ident = consts.tile([128, 128], F32)
        make_identity(nc, ident)

        # ---- input DMAs ----
        # x arrives via 4 SEPARATE tiles (one per 4-tile block) so the LN of
        # block b waits only its own chunk; weight DMAs issue from the SCALAR
        # queue (the GPSIMD queue is busy with the framework's DMA-reset +
        # sem-clear prologue for several us, which would delay the weights).
        xkvB = [
            sbBig.tile([128, 4, 128], F32, name=f"xkvB{i}") for i in range(4)
        ]
        xkv_r = xkv_d.rearrange("(p t) d -> p t d", t=NT)
        for c4 in range(4):
            nc.sync.dma_start(
                out=xkvB[c4], in_=xkv_r[:, c4 * 4 : (c4 + 1) * 4, :]
            )

        def xkv_sb(t):
            return xkvB[t // 4][:, t % 4, :]

        wq_raw = sbW.tile([D, D], F32)
        wk_raw = sbW.tile([D, D], F32)
        wv_raw = sbW.tile([D, D], F32)
        wo_raw = sbW.tile([D, D], F32)
        nc.scalar.dma_start(out=wk_raw, in_=wk_d)
        nc.scalar.dma_start(out=wq_raw, in_=wq_d)
        nc.scalar.dma_start(out=wv_raw, in_=wv_d)
        nc.scalar.dma_start(out=wo_raw, in_=wo_d)
        smallT = sbW.tile([D, 6], F32)  # cols: gamma,beta,bq,bk,bv,bo
        nc.scalar.dma_start(out=smallT, in_=vecs_d)

        # single rotating PSUM pool: tag "s" (3 bufs = 6 banks) carries
        # scores, prep, warmups and tails; ctx/den get dedicated banks.
        ps = tc.alloc_tile_pool(name="ps", bufs=1, space="PSUM")

        def rot(shape=[128, 2 * CHUNK], name="rt"):
            return ps.tile(shape, F32, name=name, tag="s", bufs=3)

        # HAM warm-up chain (independent; fills PE during DVE/DMA setup)
        def warm(n):
            for _ in range(n):
                wp = rot(name="warm")
                nc.tensor.matmul(wp[0:DH, 0:512], wones, wsrc, start=True, stop=True)

        warm(18)

        gam = smallT[:, 0:1]
        bet = smallT[:, 1:2]
        gq = sbW.tile([128, 1], F32)
        wq_f = sbW.tile([D, D], F32R)
        wk_f = sbW.tile([D, D], F32R)
        wv_f = sbW.tile([D, D], F32R)
        wo_r = sbW.tile([D, D], F32R)
        bqe = sbW.tile([128, 1], F32)
        bke = sbW.tile([128, 1], F32)
        bve = sbW.tile([128, 1], F32)
        rbias = sbW.tile([128, 1], F32)

        wkb = sbW.tile([D, D], BF16)
        wqb = sbW.tile([D, D], BF16)
        betb = sbW.tile([128, 1], BF16)

        def fold_weights():
            # gamma folds + K/Q bias columns (K first: it gates kT chunk 0).
            # Bias matmuls run in bf16 (raw fp32 weights would force slow
            # fp32 HIGH-mode weight loads on the PE ramp path).
            nc.vector.tensor_scalar_mul(wk_f, wk_raw, gam)
            nc.vector.tensor_copy(wkb, wk_raw)
            nc.vector.tensor_copy(betb, bet)
            t1 = rot(name="tb")
            nc.tensor.matmul(t1[:, 0:1], wkb, betb, start=True, stop=True)
            nc.vector.tensor_scalar_add(bke, t1[:, 0:1], smallT[:, 3:4])
            nc.vector.tensor_scalar_mul(gq, gam, float(ISQ))
            nc.vector.tensor_scalar_mul(wq_f, wq_raw, gq)
            nc.vector.tensor_scalar_mul(wv_f, wv_raw, gam)
            nc.vector.tensor_copy(wqb, wq_raw)
            t2 = rot(name="tb")
            nc.tensor.matmul(t2[:, 0:1], wqb, betb, start=True, stop=True)
            nc.vector.tensor_scalar(
                bqe, t2[:, 0:1], smallT[:, 2:3], float(ISQ), op0=OP.add, op1=OP.mult
            )

        def fold_late():
            # V/O bias columns + Wo: first needed by resid blocks (chunk 1)
            nc.vector.tensor_copy(wo_r, wo_raw)
            t3 = rot(name="tb")
            nc.tensor.matmul(t3[:, 0:1], wv_raw, bet, start=True, stop=True)
            nc.vector.tensor_scalar_add(bve, t3[:, 0:1], smallT[:, 4:5])
            t4 = rot(name="tb")
            nc.tensor.matmul(t4[:, 0:1], wo_raw, bve, start=True, stop=True)
            nc.vector.tensor_scalar_add(rbias, t4[:, 0:1], smallT[:, 5:6])

        # ---- LayerNorm + transposes + projections, pipelined with the
        # attention loop. Block b covers s-tiles 4b..4b+3: per-tile bn_stats,
        # Welford merge + bit-trick rsqrt on DVE, xn0 split DVE/ACT,
        # transposes on PE (4 into one PSUM tile, one batched ACT copy out),
        # K/Q projection bias-adds on ACT Identity.
        xn0_sb = sbBig.tile([128, NT, 128], F32)
        xkvT = sbBig.tile([128, S], F32R)  # xn0^T [d, s]
        kT = sbBig.tile([128, S], BF16)
        qT = sbBig.tile([128, QH], BF16)
        v_sb = sbBig.tile([128, NT, 128], BF16)
        residT = sbBig.tile([128, QH], F32)  # x^T + resid_bias (query half)

        def ln_block(b4, n_dve_xn0=2):
            stats = sbTmp.tile([128, 4, 6], F32, tag="st")
            for j, t in enumerate(range(b4 * 4, b4 * 4 + 4)):
                nc.vector.bn_stats(stats[:, j, :], xkv_sb(t))
            # Welford merge of bn_stats' even/odd element halves:
            # mean = m_e - 0.5*(m_e - m_o); M2 = M2e + M2o + 32*(m_e-m_o)^2
            delta = sbTmp.tile([128, 4], F32, tag="d0")
            mean4 = sbTmp.tile([128, 4], F32, tag="d1")
            m2s = sbTmp.tile([128, 4], F32, tag="d2")
            lnv = sbTmp.tile([128, 4], F32, tag="d3")
            rs4 = sbTmp.tile([128, 4], F32, tag="d4")
            nmrs = sbTmp.tile([128, 4], F32, tag="d6")
            nc.vector.tensor_tensor(
                delta, stats[:, :, 1], stats[:, :, 4], op=OP.subtract
            )
            nc.vector.scalar_tensor_tensor(
                mean4, delta, -0.5, stats[:, :, 1], op0=OP.mult, op1=OP.add
            )
            nc.vector.tensor_tensor(m2s, stats[:, :, 2], stats[:, :, 5], op=OP.add)
            nc.vector.tensor_tensor(delta, delta, delta, op=OP.mult)
            nc.vector.scalar_tensor_tensor(
                m2s, delta, 32.0, m2s, op0=OP.mult, op1=OP.add
            )
            nc.vector.tensor_scalar(
                lnv, m2s, float(1.0 / 128.0), EPS, op0=OP.mult, op1=OP.add
            )
            # rs = rsqrt(lnv): bit-trick seed + 1 Newton step
            nc.vector.tensor_scalar(
                rs4.bitcast(I32), lnv.bitcast(I32), -0.5, MAGICF,
                op0=OP.mult, op1=OP.add,
            )
            y2 = sbTmp.tile([128, 4], F32, tag="d5")
            nc.vector.tensor_tensor(y2, rs4, rs4, op=OP.mult)
            nc.vector.tensor_tensor(y2, lnv, y2, op=OP.mult)
            nc.vector.tensor_scalar(y2, y2, -0.5, 1.5, op0=OP.mult, op1=OP.add)
            nc.vector.tensor_tensor(rs4, rs4, y2, op=OP.mult)
            # nmrs = -mean*rs (bias for the ACT-Identity xn0 tiles)
            nc.vector.scalar_tensor_tensor(
                nmrs, mean4, -1.0, rs4, op0=OP.mult, op1=OP.mult
            )
            for j, t in enumerate(range(b4 * 4, b4 * 4 + 4)):
                if j < n_dve_xn0:
                    nc.vector.tensor_scalar(
                        xn0_sb[:, t, :],
                        xkv_sb(t),
                        mean4[:, j : j + 1],
                        rs4[:, j : j + 1],
                        op0=OP.subtract,
                        op1=OP.mult,
                    )
                else:
                    nc.scalar.activation(
                        xn0_sb[:, t, :], xkv_sb(t), AF.Identity,
                        bias=nmrs[:, j : j + 1], scale=rs4[:, j : j + 1],
                    )

        def proj_block(b4, fill=0):
            sl4 = slice(b4 * 4, b4 * 4 + 4)
            ps_x = rot(name="ps_x")
            for j, t in enumerate(range(b4 * 4, b4 * 4 + 4)):
                nc.tensor.transpose(
                    ps_x[:, j * 128 : (j + 1) * 128], xn0_sb[:, t, :], ident
                )
            warm(fill)
            c = b4
            nc.scalar.copy(xkvT[:, c * CHUNK : (c + 1) * CHUNK], ps_x[:, 0:512])
            pp = rot(name="ps_k")
            nc.tensor.matmul(
                pp[:, 0:512], wk_f, xkvT[:, c * CHUNK : (c + 1) * CHUNK],
                start=True, stop=True,
            )
            nc.scalar.activation(
                kT[:, c * CHUNK : (c + 1) * CHUNK], pp[:, 0:512], AF.Identity,
                bias=bke, scale=1.0,
            )
            if c < NCH:
                pp = rot(name="ps_q")
                nc.tensor.matmul(
                    pp[:, 0:512], wq_f, xkvT[:, c * CHUNK : (c + 1) * CHUNK],
                    start=True, stop=True,
                )
                nc.scalar.activation(
                    qT[:, c * CHUNK : (c + 1) * CHUNK], pp[:, 0:512], AF.Identity,
                    bias=bqe, scale=1.0,
                )
            warm(fill)
            ps_v = rot(name="ps_v")
            for j, t in enumerate(range(b4 * 4, b4 * 4 + 4)):
                nc.tensor.matmul(
                    ps_v[:, j * 128 : (j + 1) * 128],
                    xkvT[:, t * 128 : (t + 1) * 128],
                    wv_f,
                    start=True,
                    stop=True,
                )
            nc.scalar.copy(v_sb[:, sl4, :], ps_v[:, 0:512])

        def resid_block(b8):
            # 4 raw-x transposes into one PSUM tile, one biased ACT copy out
            ps_r = rot(name="ps_r")
            for j, t in enumerate(range(b8 * 4, b8 * 4 + 4)):
                nc.tensor.transpose(
                    ps_r[:, j * 128 : (j + 1) * 128], xkv_sb(t), ident
                )
            nc.scalar.activation(
                residT[:, b8 * 512 : (b8 + 1) * 512], ps_r[:, 0:512], AF.Identity,
                bias=rbias, scale=1.0,
            )

        # ---- attention ----
        pPool = tc.alloc_tile_pool(name="pPool", bufs=6)

        ctx_sb = sbBig.tile([128, CHUNK], F32)
        den_all = sbBig.tile([128, CHUNK], F32)
        ctx_ps = None
        den_ps = None

        def attn_scores(qc, kt):
            q0 = qc * CHUNK
            k0 = kt * 128
            p_sb = [None, None]
            for g, heads in enumerate(GROUPS):
                sp = ps.tile([128, 2 * CHUNK], F32, name=f"s{g}", tag="s", bufs=3)
                for i, h in enumerate(heads):
                    nc.tensor.matmul(
                        sp[:, i * CHUNK : (i + 1) * CHUNK],
                        kT[h * DH : (h + 1) * DH, k0 : k0 + 128],
                        qT[h * DH : (h + 1) * DH, q0 : q0 + CHUNK],
                        start=True,
                        stop=True,
                        tile_position=(h * DH, 0),
                    )
                if g == 0:
                    pA = pPool.tile([128, 2 * CHUNK], BF16, tag="p")
                    nc.scalar.activation(pA, sp, AF.Exp, bias=nshift, scale=1.0)
                    p_sb[0] = pA
                else:
                    pB = pPool.tile([128, 2 * CHUNK], I16, tag="p")
                    nc.vector.tensor_scalar(pB, sp, SA, SB, op0=OP.mult, op1=OP.add)
                    p_sb[1] = pB.bitcast(BF16)
            return p_sb

        def attn_ctxden(qc, kt, p_sb):
            for g, heads in enumerate(GROUPS):
                for i, h in enumerate(heads):
                    nc.tensor.matmul(
                        ctx_ps[h * DH : (h + 1) * DH, :],
                        v_sb[:, kt, h * DH : (h + 1) * DH],
                        p_sb[g][:, i * CHUNK : (i + 1) * CHUNK],
                        start=(kt == 0),
                        stop=(kt == NKT - 1),
                        tile_position=(0, h * DH),
                    )
            for g, heads in enumerate(GROUPS):
                for i, h in enumerate(heads):
                    nc.tensor.matmul(
                        den_ps[h * DH : (h + 1) * DH, :],
                        wones,
                        p_sb[g][:, i * CHUNK : (i + 1) * CHUNK],
                        start=(kt == 0),
                        stop=(kt == NKT - 1),
                        tile_position=(0, h * DH),
                    )

        def chunk_tail(qc, ctx_src, den_recip, halves=1, eng=None):
            # eng=gpsimd offloads the elementwise work to the idle GPSIMD
            # (slower but latency-tolerant for the mid-loop chunk-0 tail)
            eng = eng or nc.vector
            q0 = qc * CHUNK
            hw = CHUNK // halves
            out_pss = []
            for hh in range(halves):
                s = slice(hh * hw, (hh + 1) * hw)
                ctxn = sbTmp.tile([128, hw], F32R, tag="cn")
                eng.tensor_tensor(ctxn, ctx_src[:, s], den_recip[:, s], op=OP.mult)
                out_ps = rot(name="out_ps")
                nc.tensor.matmul(out_ps[:, 0:hw], wo_r, ctxn, start=True, stop=True)
                out_pss.append(out_ps)
            for hh in range(halves):
                fin = sbTmp.tile([128, hw], F32, tag="fin")
                nc.vector.tensor_add(
                    fin,
                    out_pss[hh][:, 0:hw],
                    residT[:, q0 + hh * hw : q0 + (hh + 1) * hw],
                )
                nc.sync.dma_start(
                    out=outT_d[:, q0 + hh * hw : q0 + (hh + 1) * hw], in_=fin
                )

        # block-0 LN (no weight dependency) before the weight folds, so the
        # DVE chain toward the first scores starts as soon as x arrives.
        ln_block(0, n_dve_xn0=4)
        fold_weights()
        proj_block(0, fill=3)

        # chunk 0: prep blocks 1-3 injected into the k-loop; scores+exp
        # emitted one ktile ahead of ctx/den (the 3-buffer rotation hides
        # the exp latency).
        ctx_ps = ps.tile([128, CHUNK], F32, name="ctx0", tag="ctx")
        den_ps = ps.tile([128, CHUNK], F32, name="den0", tag="den")
        pending = attn_scores(0, 0)
        for kt in range(NKT):
            if kt == 2:
                ln_block(1)
                proj_block(1)
            elif kt == 6:
                ln_block(2)
                proj_block(2)
            elif kt == 9:
                ln_block(3)
                proj_block(3)
            elif kt == 13:
                fold_late()
            nxt = attn_scores(0, kt + 1) if kt + 1 < NKT else None
            attn_ctxden(0, kt, pending)
            pending = nxt
        # emit chunk-1 scores before the chunk-0 ctx/den finalization so the
        # PE rolls straight into chunk 1 while the DVE copies ctx0 out
        pending = attn_scores(1, 0)
        nc.scalar.copy(ctx_sb, ctx_ps)
        nc.vector.reciprocal_approx_fast(den_all, den_ps)

        # chunk 1 (resid transposes + chunk-0 tail injected into the loop)
        ctx_ps = ps.tile([128, CHUNK], F32, name="ctx1", tag="ctx")
        den_ps = ps.tile([128, CHUNK], F32, name="den1", tag="den")
        for kt in range(NKT):
            if kt == 3:
                resid_block(0)
            elif kt == 7:
                resid_block(1)
            elif kt == 11:
                chunk_tail(0, ctx_sb, den_all, eng=nc.gpsimd)
            nxt = attn_scores(1, kt + 1) if kt + 1 < NKT else None
            attn_ctxden(1, kt, pending)
            pending = nxt
        den1r = sbTmp.tile([128, CHUNK], F32, tag="dr")
        nc.vector.reciprocal_approx_fast(den1r[:, 0:256], den_ps[:, 0:256])
        ctxn1a = sbTmp.tile([128, 256], F32R, tag="cn")
        nc.vector.tensor_tensor(
            ctxn1a, ctx_ps[:, 0:256], den1r[:, 0:256], op=OP.mult
        )
        op1a = rot(name="op1a")
        nc.tensor.matmul(op1a[:, 0:256], wo_r, ctxn1a, start=True, stop=True)
        nc.vector.reciprocal_approx_fast(den1r[:, 256:512], den_ps[:, 256:512])
        ctxn1b = sbTmp.tile([128, 256], F32R, tag="cn")
        nc.vector.tensor_tensor(
            ctxn1b, ctx_ps[:, 256:512], den1r[:, 256:512], op=OP.mult
        )
        op1b = rot(name="op1b")
        nc.tensor.matmul(op1b[:, 0:256], wo_r, ctxn1b, start=True, stop=True)
        fin1a = sbTmp.tile([128, 256], F32, tag="fin")
        nc.vector.tensor_add(fin1a, op1a[:, 0:256], residT[:, 512:768])
        nc.sync.dma_start(out=outT_d[:, 512:768], in_=fin1a)
        fin1b = sbTmp.tile([128, 256], F32, tag="fin")
        nc.vector.tensor_add(fin1b, op1b[:, 0:256], residT[:, 768:1024])
        nc.sync.dma_start(out=outT_d[:, 768:1024], in_=fin1b)

        pPool.release()
        ps.release()
        sbTmp.release()
        sbBig.release()
        sbW.release()
        consts.release()

    nc.compile()
    return nc


def _get_compiled():
    global _compiled
    if _compiled is None:
        _compiled = _build()
    return _compiled


# device position j <- host row (j%128)*16 + j//128
_DEV2HOST = (np.arange(S) % 128) * NT + np.arange(S) // 128
_HOSTPERM = np.empty(S, dtype=np.int64)
_HOSTPERM[_DEV2HOST] = np.arange(S)


def kernel(x, Wq, bq, Wk, bk, Wv, bv, gamma, beta, Wo, bo):
    x = np.asarray(x, dtype=np.float32)
    vecs = np.ascontiguousarray(
        np.stack(
            [np.asarray(a, dtype=np.float32) for a in (gamma, beta, bq, bk, bv, bo)]
        ).T
    )
    wq = np.ascontiguousarray(np.asarray(Wq, dtype=np.float32))
    wk = np.ascontiguousarray(np.asarray(Wk, dtype=np.float32))
    wv = np.ascontiguousarray(np.asarray(Wv, dtype=np.float32))
    wo = np.ascontiguousarray(np.asarray(Wo, dtype=np.float32))

    nc = _get_compiled()

    in_maps = []
    for c in range(N_CORES):
        b, half = c // 2, c % 2
        off = half * QH
        xroll = np.roll(x[b], -off, axis=0)
        xin = np.ascontiguousarray(xroll[_HOSTPERM])
        in_maps.append(
            {"xkv": xin, "wq": wq, "wk": wk, "wv": wv, "wo": wo, "vecs": vecs}
        )

    res = run_bass_kernel_spmd(nc, in_maps, core_ids=list(range(N_CORES)), trace=False)

    out = np.empty((B, S, D), dtype=np.float32)
    for c in range(N_CORES):
        b, half = c // 2, c % 2
        off = half * QH
        out[b, off : off + QH, :] = res.results[c]["outT"].T
    return out
